# revision 1
# baseline (speedup 1.0000x reference)
"""Trainium2 Bass kernel for nn_CP_Based (CP-decomposition interaction layer).

Math (full problem):
    t[b,f,r,u] = sum_d X[b,f,d] * K[d,r,f,u]      (B=1024, F=64, D=4, R=32, U=128)
    had[b,r,u] = prod_f t[b,f,r,u]
    out[b,u]   = sum_r had[b,r,u]

Strategy:
  * Shard batch across 8 cores (B_loc = 128 = one partition tile).
  * Feature-tripling (host-side weight repack): for a triple (f0,f1,f2),
        t3 = t[.,f0,.] * t[.,f1,.] * t[.,f2,.]
           = sum_{d3=0..63} X3[b,j,d3] * K3[d3,r,j,u]
    with X3/K3 outer products of the per-feature slices. One K=64 matmul per
    triple replaces three K=4 matmuls AND cuts the elementwise hadamard from
    63 to 21 multiplies per output element (the DVE is the bottleneck engine:
    fp32 tensor_tensor runs at 1 elem/cycle/partition @ 0.96 GHz).
    64 = 21*3 + 1: factor 21 is the lone feature 63, zero-padded to K=64.
  * PE: 2 factors run concurrently via row tiling (tile_position=(64s,0)),
    each filling a [128,1024] 2-bank psum tile per (r,u) chunk.
  * DVE: running product P[b, r*u] *= psum factor chunks (one PSUM operand
    per op is a HW limit). ScalarE initializes P for the first factor.
  * Final sum over r: strided tensor_reduce.

Host prep is pure input repacking (outer products of the small inputs,
~12M mults vs ~1.3G MACs + 270M multiplies on device).
"""

import numpy as np

B, F, D, R, U = 1024, 64, 4, 32, 128
NCORES = 8
BLOC = B // NCORES          # 128 batch rows per core
NFAC = 22                   # 21 triples + 1 padded single
NGRP = NFAC // 2            # 11 groups of 2 row-tiled factors
D3 = 64                     # contraction dim per triple (4^3)
RU = R * U                  # 4096
CHUNK = 1024                # 2 psum banks per factor-chunk
NCHUNK = RU // CHUNK        # 4

_cached = {}


def _build_nc(n_rep=1, chunk=CHUNK, unroll_reps=False):
    import concourse.bass as bass
    import concourse.mybir as mybir
    import concourse.tile as tile
    from concourse import bacc

    nch = RU // chunk
    nps = 8 // (chunk // 512)  # psum tiles to fill all 8 banks
    fp32 = mybir.dt.float32
    nc = bacc.Bacc("TRN2", target_bir_lowering=False, debug=False)

    xt_d = nc.dram_tensor("xt", [128, NGRP * BLOC], fp32, kind="ExternalInput").ap()
    kr_d = nc.dram_tensor("kr", [NGRP, 128, RU], fp32, kind="ExternalInput").ap()
    out_d = nc.dram_tensor("out", [BLOC, U], fp32, kind="ExternalOutput").ap()

    with tile.TileContext(nc) as tc:
        with (
            tc.tile_pool(name="const", bufs=1) as const_pool,
            tc.tile_pool(name="kt", bufs=3) as kpool,
            tc.tile_pool(name="prod", bufs=1) as ppool,
            tc.tile_pool(name="outp", bufs=1) as opool,
            tc.tile_pool(name="ps", bufs=nps, space="PSUM") as pspool,
        ):
            xt = const_pool.tile([128, NGRP * BLOC], fp32)
            nc.sync.dma_start(xt[:], xt_d[:])

            P = ppool.tile([128, RU], fp32)

            def body():
                for m in range(NGRP):
                    kt = kpool.tile([128, RU], fp32, tag="kt")
                    nc.sync.dma_start(kt[:], kr_d[m])
                    for c in range(nch):
                        sl = slice(c * chunk, (c + 1) * chunk)
                        ps = []
                        for s in range(2):
                            pst = pspool.tile([128, chunk], fp32, tag="ps")
                            for h in range(chunk // 512):
                                hs = slice(h * 512, (h + 1) * 512)
                                nc.tensor.matmul(
                                    pst[:, hs],
                                    xt[64 * s : 64 * s + D3, m * BLOC : (m + 1) * BLOC],
                                    kt[64 * s : 64 * s + D3, c * chunk + h * 512 : c * chunk + (h + 1) * 512],
                                    start=True,
                                    stop=True,
                                    tile_position=(64 * s, 0),
                                )
                            ps.append(pst)
                        # DVE reads at most one PSUM operand per op: chain the
                        # running product through SBUF. Init via ScalarE copy.
                        if m == 0:
                            nc.scalar.copy(P[:, sl], ps[0][:])
                        else:
                            nc.vector.tensor_mul(P[:, sl], P[:, sl], ps[0][:])
                        nc.vector.tensor_mul(P[:, sl], P[:, sl], ps[1][:])

            if n_rep == 1:
                body()
            elif unroll_reps:
                for _ in range(n_rep):
                    body()
            else:
                # benchmarking mode: repeat the (idempotent) body on-device
                with tc.For_i(0, n_rep, 1):
                    body()

            osum = opool.tile([BLOC, U], fp32)
            nc.vector.tensor_reduce(
                osum[:],
                P[:].rearrange("p (r u) -> p u r", r=R),
                axis=mybir.AxisListType.X,
                op=mybir.AluOpType.add,
            )
            nc.sync.dma_start(out_d[:], osum[:])

    nc.compile()
    return nc


def _host_prep(X, K):
    """Repack inputs: per-core X3 outer products + shared K3 outer products.

    Factor j < 21 covers features (3j, 3j+1, 3j+2) with contraction index
    d3 = 16*d0 + 4*d1 + d2; factor 21 is feature 63 (d3 = d, rest zero).
    Packed layouts match SBUF tiles directly:
      kr[m, row, r*U+u]: row = 64*s + d3 holds factor (2m+s).
      xt[row, m*BLOC+b]: same row convention.
    """
    f32 = np.float32
    NT = 21
    fa = [3 * j for j in range(NT)]

    # K3 [j, d3, r*u]
    ka = K[:, :, [3 * j for j in range(NT)], :]      # [4, 32, 21, 128] (d,r,j,u)
    kb = K[:, :, [3 * j + 1 for j in range(NT)], :]
    kc = K[:, :, [3 * j + 2 for j in range(NT)], :]
    K3 = (
        ka[:, None, None] * kb[None, :, None] * kc[None, None, :]
    )                                                # [4,4,4,32,21,128] (d0,d1,d2,r,j,u)
    K3 = K3.transpose(4, 0, 1, 2, 3, 5).reshape(NT, D3, RU)  # [j, d3, r*u]
    K3f = np.zeros((NFAC, D3, RU), dtype=f32)
    K3f[:NT] = K3
    K3f[NT, :D, :] = K[:, :, 63, :].reshape(D, RU)   # lone feature 63
    kr = np.ascontiguousarray(
        K3f.reshape(NGRP, 2, D3, RU).reshape(NGRP, 128, RU)
    )

    # X3 per core [row, m*BLOC+b]
    xts = []
    for c in range(NCORES):
        Xc = X[c * BLOC : (c + 1) * BLOC]            # [128, 64, 4] (b, f, d)
        xa = Xc[:, [3 * j for j in range(NT)], :]    # [b, j, 4]
        xb = Xc[:, [3 * j + 1 for j in range(NT)], :]
        xc = Xc[:, [3 * j + 2 for j in range(NT)], :]
        X3 = (
            xa[:, :, :, None, None] * xb[:, :, None, :, None] * xc[:, :, None, None, :]
        )                                            # [b, j, 4, 4, 4]
        X3 = X3.reshape(BLOC, NT, D3)
        X3f = np.zeros((BLOC, NFAC, D3), dtype=f32)
        X3f[:, :NT] = X3
        X3f[:, NT, :D] = Xc[:, 63, :]
        xt = X3f.transpose(1, 2, 0).reshape(NGRP, 128, BLOC)  # [m, row, b]
        xts.append(np.ascontiguousarray(xt.transpose(1, 0, 2).reshape(128, NGRP * BLOC)))
    return xts, kr


def kernel(**inputs):
    from concourse.bass_utils import run_bass_kernel_spmd

    X = np.asarray(inputs["X"], dtype=np.float32)
    K = np.asarray(inputs["kernel"], dtype=np.float32)
    assert X.shape == (B, F, D) and K.shape == (D, R, F, U)

    if "nc" not in _cached:
        _cached["nc"] = _build_nc()
    nc = _cached["nc"]

    xts, kr = _host_prep(X, K)
    in_maps = [{"xt": xts[c], "kr": kr} for c in range(NCORES)]
    res = run_bass_kernel_spmd(nc, in_maps, core_ids=list(range(NCORES)))
    return np.concatenate([res.results[c]["out"] for c in range(NCORES)], axis=0)



# revision 9
# speedup vs baseline: 1.0533x; 1.0533x over previous
"""Trainium2 Bass kernel for nn_CP_Based (CP-decomposition interaction layer).

Math (full problem):
    t[b,f,r,u] = sum_d X[b,f,d] * K[d,r,f,u]      (B=1024, F=64, D=4, R=32, U=128)
    had[b,r,u] = prod_f t[b,f,r,u]
    out[b,u]   = sum_r had[b,r,u]

Strategy (v1):
  * Feature-tripling (host-side repack): triple (f0,f1,f2) -> one K=64
    contraction: t3 = sum_{d3} X3[b,d3] * K3[d3,r,u], d3 = 4^3.
    21 triples + 1 padded single = 22 factor planes; hadamard muls drop
    from 63 to 21 per output element.
  * Sharding 4x2: batch/4 x units/2 per core. Cuts per-core HBM traffic
    to X3 (1.4MB) + K3 (11.5MB) vs 23.8MB for batch-only sharding.
  * Matmuls in float32r: 1 cycle/row (vs 4 for fp32) when free dim>=256.
    Two factors co-execute on disjoint PE row-halves (tile_position).
  * Column order u-major/r-inner so the final sum over r is a contiguous
    innermost-axis tensor_reduce.
  * Elementwise product pipeline over three engines:
      - DVE multiplies factor planes directly from PSUM (only engine
        that can, at full rate),
      - ScalarE (Act) copies a subset of planes PSUM->SBUF,
      - Pool (gpsimd; no PSUM port) merges the copied planes in SBUF.
    Join + r-reduce at the end per batch tile.
"""

import numpy as np

B, F, D, R, U = 1024, 64, 4, 32, 128
NCORES = 8
NB, NU = 4, 2                # core grid: 4 batch shards x 2 unit shards
BLOC = B // NB               # 256 batch rows per core
NBT = BLOC // 128            # 2 partition tiles of batch
ULOC = U // NU               # 64 units per core
RULOC = ULOC * R             # 2048 columns per core (u-major, r-inner)
NT = 21                      # feature triples
NFAC = 22                    # 21 triples + 1 padded single
NGRP = NFAC // 2             # 11 pairs of row-tiled factors
D3 = 64                      # contraction dim per triple

# Planes consumed by Act->Pool branch (SBUF tree); rest go to the DVE
# PSUM chain. Interleaved with DVE planes so Act and DVE run concurrently.
A_PLANES = (1, 3, 5, 7, 9, 11, 13, 15)

_cached = {}


def _build_nc():
    import concourse.bass as bass
    import concourse.mybir as mybir
    import concourse.tile as tile
    from concourse import bacc

    fp32 = mybir.dt.float32
    fp32r = mybir.dt.float32r
    nc = bacc.Bacc("TRN2", target_bir_lowering=False, debug=False)

    xt_d = nc.dram_tensor("xt", [128, NGRP * BLOC], fp32r, kind="ExternalInput").ap()
    kr_d = nc.dram_tensor("kr", [NGRP, 128, RULOC], fp32r, kind="ExternalInput").ap()
    out_d = nc.dram_tensor("out", [BLOC, ULOC], fp32, kind="ExternalOutput").ap()

    a_set = set(A_PLANES)

    with tile.TileContext(nc) as tc:
        with (
            tc.tile_pool(name="const", bufs=1) as const_pool,
            tc.tile_pool(name="kt", bufs=NGRP) as kpool,
            tc.tile_pool(name="pd", bufs=1) as pdpool,
            tc.tile_pool(name="sc", bufs=4) as spool,
            tc.tile_pool(name="tm", bufs=3) as tpool,
            tc.tile_pool(name="pj", bufs=2) as pjpool,
            tc.tile_pool(name="outp", bufs=1) as opool,
            tc.tile_pool(name="ps", bufs=2, space="PSUM") as pspool,
        ):
            xt = const_pool.tile([128, NGRP * BLOC], fp32r)
            nc.sync.dma_start(xt[:], xt_d[:])

            # Whole (sharded) kernel stays resident: 11 tiles x 8KB/partition.
            kts = []
            for m in range(NGRP):
                kt = kpool.tile([128, RULOC], fp32r, tag="kt")
                nc.sync.dma_start(kt[:], kr_d[m])
                kts.append(kt)

            P_d = pdpool.tile([128, NBT * RULOC], fp32)
            osum = opool.tile([128, NBT * ULOC], fp32)

            for bt in range(NBT):
                pdsl = slice(bt * RULOC, (bt + 1) * RULOC)
                scp = {}       # A-plane j -> sbuf tile
                merge_q = []   # pending pool-tree roots
                nmerge = 0

                def pool_merge(t_new):
                    # eager binary tree merge of copied planes
                    nonlocal nmerge
                    merge_q.append(t_new)
                    while len(merge_q) >= 2 and (len(merge_q) % 2 == 0):
                        b_ = merge_q.pop()
                        a_ = merge_q.pop()
                        dst = tpool.tile([128, RULOC], fp32, tag="tm")
                        nc.gpsimd.tensor_mul(dst[:], a_[:], b_[:])
                        nmerge += 1
                        merge_q.append(dst)

                first_d = True
                for m in range(NGRP):
                    kt = kts[m]
                    pss = []
                    for s in range(2):
                        ps = pspool.tile([128, RULOC], fp32, tag="ps")
                        pss.append(ps)
                    # interleave the two row-tiles' matmuls for co-execution
                    for c in range(RULOC // 512):
                        cs = slice(c * 512, (c + 1) * 512)
                        for s in range(2):
                            nc.tensor.matmul(
                                pss[s][:, cs],
                                xt[64 * s : 64 * s + D3,
                                   m * BLOC + bt * 128 : m * BLOC + (bt + 1) * 128],
                                kt[64 * s : 64 * s + D3, cs],
                                start=True,
                                stop=True,
                                tile_position=(64 * s, 0),
                            )
                    for s in range(2):
                        j = 2 * m + s
                        ps = pss[s]
                        if j in a_set:
                            sc = spool.tile([128, RULOC], fp32, tag="sc")
                            nc.scalar.copy(sc[:], ps[:])
                            scp[j] = sc
                            pool_merge(sc)
                        elif first_d:
                            nc.scalar.copy(P_d[:, pdsl], ps[:])
                            first_d = False
                        else:
                            # two half-plane ops -> independent dep chains on DVE
                            for h in range(2):
                                hs = slice(bt * RULOC + h * 1024,
                                           bt * RULOC + (h + 1) * 1024)
                                phs = slice(h * 1024, (h + 1) * 1024)
                                nc.vector.tensor_mul(P_d[:, hs], P_d[:, hs], ps[:, phs])

                # collapse any leftover pool-tree roots
                while len(merge_q) > 1:
                    b_ = merge_q.pop()
                    a_ = merge_q.pop()
                    dst = tpool.tile([128, RULOC], fp32, tag="tm")
                    nc.gpsimd.tensor_mul(dst[:], a_[:], b_[:])
                    nmerge += 1
                    merge_q.append(dst)
                p_pool = merge_q.pop()

                # join on Pool (SBUF x SBUF), then contiguous r-reduce on DVE
                pj = pjpool.tile([128, RULOC], fp32, tag="pj")
                nc.gpsimd.tensor_mul(pj[:], P_d[:, pdsl], p_pool[:])
                nc.vector.tensor_reduce(
                    osum[:, bt * ULOC : (bt + 1) * ULOC],
                    pj[:].rearrange("p (u r) -> p u r", r=R),
                    axis=mybir.AxisListType.X,
                    op=mybir.AluOpType.add,
                )

            for bt in range(NBT):
                nc.sync.dma_start(
                    out_d[bt * 128 : (bt + 1) * 128, :],
                    osum[:, bt * ULOC : (bt + 1) * ULOC],
                )

    nc.compile()
    return nc


def _host_prep(X, K):
    """Repack inputs per core.

    Factor j < 21 covers features (3j, 3j+1, 3j+2), contraction index
    d3 = 16*d0 + 4*d1 + d2; factor 21 is feature 63 zero-padded.
    Row convention: row = 64*s + d3 holds factor j = 2m+s.
    Column convention (kernel side): col = u_loc*32 + r  (u-major, r-inner).

    xt[core][row, m*BLOC + bt*128 + b]
    kr[uq][m, row, u_loc*32 + r]
    """
    f32 = np.float32
    ia = [3 * j for j in range(NT)]
    ib = [3 * j + 1 for j in range(NT)]
    ic = [3 * j + 2 for j in range(NT)]

    # K3 [j, d3, r, u]
    ka = K[:, :, ia, :].astype(np.float64)    # [4, 32, 21, 128] (d,r,j,u)
    kb = K[:, :, ib, :].astype(np.float64)
    kc = K[:, :, ic, :].astype(np.float64)
    K3 = (
        ka[:, None, None] * kb[None, :, None] * kc[None, None, :]
    )                                          # [d0,d1,d2,r,j,u]
    K3 = K3.transpose(4, 0, 1, 2, 3, 5).reshape(NT, D3, R, U)
    K3f = np.zeros((NFAC, D3, R, U), dtype=np.float64)
    K3f[:NT] = K3
    K3f[NT, :D] = K[:, :, 63, :]
    # u-major, r-inner columns + pack factor pairs into 128 rows
    K3f = K3f.transpose(0, 1, 3, 2)            # [j, d3, u, r]
    krs = []
    for uq in range(NU):
        sl = K3f[:, :, uq * ULOC : (uq + 1) * ULOC, :].reshape(NFAC, D3, RULOC)
        krs.append(
            np.ascontiguousarray(
                sl.reshape(NGRP, 2 * D3, RULOC), dtype=f32
            )
        )

    # X3 per batch shard
    xts = []
    for cb in range(NB):
        Xc = X[cb * BLOC : (cb + 1) * BLOC].astype(np.float64)  # [256, 64, 4]
        xa = Xc[:, ia, :]
        xb = Xc[:, ib, :]
        xc = Xc[:, ic, :]
        X3 = (
            xa[:, :, :, None, None] * xb[:, :, None, :, None] * xc[:, :, None, None, :]
        ).reshape(BLOC, NT, D3)
        X3f = np.zeros((BLOC, NFAC, D3), dtype=np.float64)
        X3f[:, :NT] = X3
        X3f[:, NT, :D] = Xc[:, 63, :]
        xt = X3f.transpose(1, 2, 0).reshape(NGRP, 128, BLOC)   # [m, row, b]
        xts.append(
            np.ascontiguousarray(
                xt.transpose(1, 0, 2).reshape(128, NGRP * BLOC), dtype=f32
            )
        )
    return xts, krs


def kernel(**inputs):
    from concourse.bass_utils import run_bass_kernel_spmd

    X = np.asarray(inputs["X"], dtype=np.float32)
    K = np.asarray(inputs["kernel"], dtype=np.float32)
    assert X.shape == (B, F, D) and K.shape == (D, R, F, U)

    if "nc" not in _cached:
        _cached["nc"] = _build_nc()
    nc = _cached["nc"]

    xts, krs = _host_prep(X, K)
    in_maps = [
        {"xt": xts[c // NU], "kr": krs[c % NU]} for c in range(NCORES)
    ]
    res = run_bass_kernel_spmd(nc, in_maps, core_ids=list(range(NCORES)))
    out = np.empty((B, U), dtype=np.float32)
    for c in range(NCORES):
        cb, uq = c // NU, c % NU
        out[cb * BLOC : (cb + 1) * BLOC, uq * ULOC : (uq + 1) * ULOC] = (
            res.results[c]["out"]
        )
    return out
